# revision 11
# baseline (speedup 1.0000x reference)
"""Trainium2 Bass kernel for nn_CoTLayer (CoT attention layer).

Self-contained: takes FULL inputs (as produced by setup_inputs()), shards
batch across 8 NeuronCores (data-parallel, weights replicated), runs a
Bass/Tile kernel per core, returns the FULL output.

Per-core layout ("dual-half" packing, one batch image per core):
  - partitions p = c + 64*half: channels 0..63 of the TOP image half on
    partitions 0..63, same channels of the BOTTOM half on 64..127 -> every
    engine pass runs 128 partitions wide at half the free-dim length.
  - x and v stored zero-padded (128, PH2, PW) so 3x3 taps are strided AP
    reads; one halo row per half is computed redundantly.
  - k, w1, vAgg stored dense; matmuls stream interior-only 3D APs, so no
    pad garbage pollutes GN statistics or the global average pool.
"""

import numpy as np
import ml_dtypes
from contextlib import ExitStack

import concourse.bass as bass
import concourse.tile as tile
from concourse import bacc, mybir

F32 = mybir.dt.float32
F32R = mybir.dt.float32r
BF16 = mybir.dt.bfloat16
AF = mybir.ActivationFunctionType
OP = mybir.AluOpType


# float32r (TF32-like) matmuls run 4x faster than fp32 on the PE at
# free-size >= 256. The BIR verifier requires every producer of an f32r
# matmul operand to write f32r (DMA/Act/DVE all qualify; Memset does not),
# so the big matmul-feeding tiles are declared f32r and their DMAs bitcast.
R_TILES = {"kwd", "w1wx", "c1td", "e3272d"}
BF_TILES = {"w1wk"}

B, DIM, H, W = 8, 64, 128, 128
KS, K2, SP, RADIX = 3, 9, 8, 2
ATTN = 32
EMB = 72
G = DIM // SP          # 8 groups
GS = EMB // G          # 9 channels per group
EPS = 1e-5
N_CORES = 8

TAPS = [(ki, kj) for ki in range(KS) for kj in range(KS)]  # t = ki*3+kj


# ---------------------------------------------------------------- host prep

def _blockdiag2(a):
    """(k, m) -> (2k, 2m) with a on both diagonal blocks."""
    k, m = a.shape
    out = np.zeros((2 * k, 2 * m), np.float32)
    out[:k, :m] = a
    out[k:, m:] = a
    return out


def prep_weights(inp):
    """Fold BN scales into weights host-side; build matmul-ready tensors."""
    f = np.float32
    sc = 1.0 / np.sqrt(f(1.0 + EPS))

    ke_w = np.asarray(inp["ke_w"], f)            # (64, 16, 3, 3)
    sck = np.asarray(inp["ke_g"], f) * sc        # (64,)
    kwd = np.zeros((2 * DIM, K2 * 2 * DIM), f)   # dual lhsT per tap
    for t, (ki, kj) in enumerate(TAPS):
        blk = np.zeros((DIM, DIM), f)            # (in, out)
        for o in range(DIM):
            g = o // 16
            blk[16 * g:16 * g + 16, o] = ke_w[o, :, ki, kj] * sck[o]
        kwd[:, t * 2 * DIM:(t + 1) * 2 * DIM] = _blockdiag2(blk)

    sc1 = np.asarray(inp["em_g1"], f) * sc       # (32,)
    em_w1 = np.asarray(inp["em_w1"], f)          # (32, 128)
    w1w = (em_w1 * sc1[:, None]).T.copy()        # (128, 32): rows 0-63 x part
    w1wx = _blockdiag2(w1w[:DIM])                # (128, 64)
    w1wk = _blockdiag2(w1w[DIM:])                # (128, 64)

    scc = np.asarray(inp["c1_g"], f) * sc
    c1t = (np.asarray(inp["c1_w"], f) * scc[:, None]).T.copy()   # (64, 64)
    c1td = _blockdiag2(c1t)                      # (128, 128)
    c1th = np.vstack([c1t, c1t])                 # (128, 64) for halo rows

    em_w2 = np.asarray(inp["em_w2"], f)          # (72, 32)
    e3272d = np.vstack([em_w2.T, em_w2.T]).copy()  # (64, 72)
    e72 = em_w2.copy()                           # (72, 32)

    sc_bn = np.asarray(inp["bn_g"], f) * sc      # (64,) silu-BN scale
    sel = np.zeros((EMB, K2 * DIM), f)
    for t in range(K2):
        for c in range(DIM):
            sel[(c % SP) * GS + t, t * DIM + c] = sc_bn[c]

    b2 = np.asarray(inp["em_b2"], f)             # (72,)
    g8 = np.zeros((EMB, G), f)
    gb8 = np.zeros((EMB, G), f)
    x8to72 = np.zeros((G, EMB), f)
    for e in range(EMB):
        g8[e, e // GS] = 1.0 / GS
        gb8[e, e // GS] = b2[e] / GS
        x8to72[e // GS, e] = 1.0
    mbg = np.array([b2[9 * g:9 * g + 9].mean() for g in range(G)], f)[:, None]
    b2sq = np.array([(b2[9 * g:9 * g + 9] ** 2).mean() for g in range(G)], f)[:, None]

    scse = np.asarray(inp["se_g"], f) * sc
    sew1 = (np.asarray(inp["se_w1"], f) * scse[:, None]).T.copy()   # (64, 32)
    seb1 = (np.asarray(inp["se_b1"], f) * scse + np.asarray(inp["se_bb"], f))[:, None]
    sew2 = np.asarray(inp["se_w2"], f).T.copy()                      # (32, 128)

    dif = np.zeros((RADIX * DIM, DIM), f)
    for m in range(DIM):
        dif[2 * m, m] = 1.0
        dif[2 * m + 1, m] = -1.0

    fold2 = np.vstack([np.eye(DIM, dtype=f), np.eye(DIM, dtype=f)])  # (128, 64)

    def dup(v):   # (64,1)->(128,1)
        return np.vstack([v, v]).astype(f)

    keb = np.asarray(inp["ke_b"], f)[:, None]
    c1b = np.asarray(inp["c1_b"], f)[:, None]
    w1b = np.asarray(inp["em_b1"], f)[:, None]
    bnb = np.asarray(inp["bn_b"], f)[:, None]

    return {
        "kwd": kwd,
        "kebd": dup(keb),
        "w1wx": w1wx,
        "w1wk": w1wk.astype(ml_dtypes.bfloat16),
        "w1bd": np.vstack([w1b, w1b]).astype(f),     # (64, 1)
        "c1td": c1td,
        "c1th": c1th,
        "c1bd": dup(c1b),
        "e3272d": e3272d,
        "e72": e72,
        "sel": sel,
        "g8": g8,
        "gb8": gb8,
        "x8to72": x8to72,
        "mbg": mbg,
        "b2sq": b2sq,
        "b2c": b2[:, None].copy(),
        "gng": np.asarray(inp["em_gn_g"], f)[:, None],
        "gnb": np.asarray(inp["em_gn_b"], f)[:, None],
        "bnbd": dup(bnb),
        "fold2": fold2,
        "sew1": sew1,
        "seb1": seb1,
        "sew2": sew2,
        "seb2": np.asarray(inp["se_b2"], f)[:, None],
        "dif": dif,
        "i128": np.eye(2 * DIM, dtype=f),
        "ones1": np.ones((1, 2 * DIM), f),
    }


def pack_x(xb, h, w):
    """(64, h, w) -> dual-half padded (128, HH+2, w+4)."""
    HH = h // 2
    PH2, PW = HH + 2, w + 4
    xp = np.zeros((2 * DIM, PH2, PW), np.float32)
    xb = np.asarray(xb, np.float32)
    # half A: buffer row r = image row r-1 (rows 1..HH+1 = image 0..HH)
    xp[:DIM, 1:PH2, 1:w + 1] = xb[:, 0:HH + 1, :]
    # half B: buffer row r = image row HH-1+r (rows 0..HH = image HH-1..h-1)
    xp[DIM:, 0:PH2 - 1, 1:w + 1] = xb[:, HH - 1:h, :]
    return xp


# ---------------------------------------------------------------- device code

def build_nc(h=H, w=W):
    """Build the per-core Bass program (parametric spatial size for testing)."""
    HH = h // 2
    PH2, PW = HH + 2, w + 4
    R = max(1, min(HH, 512 // w))     # image rows per tile (per half)
    assert HH % R == 0
    NT = R * w                        # matmul free size per tile (<=512)
    ntiles = HH // R
    HW = h * w
    HW2 = HH * w

    nc = bacc.Bacc("TRN2", target_bir_lowering=False, debug=False,
                   num_devices=N_CORES)

    dp = nc.declare_dram_parameter
    xd = dp("x", [2 * DIM, PH2, PW], F32, isOutput=False)
    names = {
        "kwd": [2 * DIM, K2 * 2 * DIM], "kebd": [2 * DIM, 1],
        "w1wx": [2 * DIM, 2 * ATTN], "w1wk": [2 * DIM, 2 * ATTN],
        "w1bd": [2 * ATTN, 1], "c1td": [2 * DIM, 2 * DIM],
        "c1th": [2 * DIM, DIM], "c1bd": [2 * DIM, 1],
        "e3272d": [2 * ATTN, EMB], "e72": [EMB, ATTN], "sel": [EMB, K2 * DIM],
        "g8": [EMB, G], "gb8": [EMB, G], "x8to72": [G, EMB],
        "mbg": [G, 1], "b2sq": [G, 1], "b2c": [EMB, 1], "gng": [EMB, 1],
        "gnb": [EMB, 1], "bnbd": [2 * DIM, 1], "fold2": [2 * DIM, DIM],
        "sew1": [DIM, ATTN], "seb1": [ATTN, 1],
        "sew2": [ATTN, RADIX * DIM], "seb2": [RADIX * DIM, 1],
        "dif": [RADIX * DIM, DIM], "i128": [2 * DIM, 2 * DIM],
        "ones1": [1, 2 * DIM],
    }
    wd = {k: dp(k, s, BF16 if k in BF_TILES else F32, isOutput=False)
          for k, s in names.items()}
    od = dp("out", [DIM, h, w], F32, isOutput=True)

    with tile.TileContext(nc) as tc, ExitStack() as ctx:
        wp = ctx.enter_context(tc.tile_pool(name="wp", bufs=1))
        bigp = ctx.enter_context(tc.tile_pool(name="bigp", bufs=1))
        wkp = ctx.enter_context(tc.tile_pool(name="wkp", bufs=2))

        # --- persistent SBUF ---
        # x is split into NXT row-band tiles so phase-1 compute on band 0 can
        # start as soon as its DMA lands instead of waiting for the full load.
        TPX = min(4, ntiles)              # compute tiles per x band
        assert ntiles % TPX == 0
        NXT = ntiles // TPX
        XROWS = TPX * R + 2               # band rows incl. 3x3 halo
        XTs = [bigp.tile([128, XROWS * PW], F32R, name=f"xt{j}")
               for j in range(NXT)]
        xvs = [t_.rearrange("p (r c) -> p r c", c=PW) for t_ in XTs]
        VPB = bigp.tile([128, PH2 * PW], BF16)        # padded v
        KB = bigp.tile([128, HW2], BF16)              # dense k
        W1B = bigp.tile([2 * ATTN, HW2], F32R)        # dense w1 (A rows 0-31)
        VAGG = bigp.tile([128, HW2], F32R)            # silu'd aggregation
        vv = VPB.rearrange("p (r c) -> p r c", c=PW)

        def _wdt(k):
            if k in R_TILES:
                return F32R
            return BF16 if k in BF_TILES else F32
        wt = {k: wp.tile(list(v.shape), _wdt(k), name=f"w_{k}")
              for k, v in wd.items()}
        zbias = wp.tile([128, 1], F32)
        nc.vector.memset(zbias[:], 0.0)
        epst = wp.tile([G, 1], F32)
        nc.vector.memset(epst[:], EPS)
        stats6 = wp.tile([EMB, 12 * ntiles], F32)
        ksums = wp.tile([2 * DIM, ntiles], F32)
        vsums = wp.tile([2 * DIM, ntiles], F32)
        LT2 = wp.tile([2 * ATTN, K2 * 2 * DIM], F32R)
        LT2v = LT2.rearrange("p (t c) -> p t c", c=2 * DIM)
        BT2 = wp.tile([2 * DIM, K2], F32)
        diag0 = wp.tile([2 * DIM, 2 * DIM], F32R)
        diag1 = wp.tile([2 * DIM, 2 * DIM], BF16)

        # --- load x band 0, weights, then remaining bands (in compute order)
        def load_band(j):
            base = TPX * R * j
            ch = max(1, XROWS // 3)
            for r in range(0, XROWS, ch):
                r2 = min(XROWS, r + ch)
                nc.sync.dma_start(
                    out=xvs[j][:, r:r2, :],
                    in_=xd[:, base + r:base + r2, :].bitcast(F32R))
        first = ["kwd", "c1th", "c1td", "w1wx", "w1wk", "kebd", "c1bd",
                 "w1bd", "e3272d"]
        def load_w(k):
            src = wd[k][:].bitcast(F32R) if k in R_TILES else wd[k][:]
            nc.sync.dma_start(out=wt[k][:], in_=src)
        for k in first:
            load_w(k)
        load_band(0)
        for k in wt:
            if k not in first:
                load_w(k)
        for j in range(1, NXT):
            load_band(j)
        # zero the off-diagonal blocks of LT2 once (Act writes f32r; a plain
        # memset would not satisfy the f32r-producer rule)
        nc.scalar.activation(
            LT2v[0:ATTN, :, DIM:2 * DIM],
            wt["sel"][0:ATTN, :].rearrange("p (t c) -> p t c", c=DIM),
            AF.Copy, scale=0.0)
        nc.gpsimd.memset(vv[:, 0:1, :], 0.0)
        nc.gpsimd.memset(vv[:, PH2 - 1:PH2, :], 0.0)
        nc.gpsimd.memset(vv[:, 1:PH2 - 1, 0:1], 0.0)
        nc.gpsimd.memset(vv[:, 1:PH2 - 1, w + 1:PW], 0.0)

        def r3(ap2d):  # dense (p, NT) -> (p, R, w)
            return ap2d.rearrange("p (r c) -> p r c", c=w)

        # ---------------- phase 1: k, w1, v, GN stats ----------------
        with tc.tile_pool(name="ps1", bufs=2, space="PSUM") as ps1:
            # v halo rows (one per half), computed redundantly
            hps_a = ps1.tile([DIM, w], F32, tag="vps")
            nc.tensor.matmul(hps_a[:], wt["c1th"][0:DIM, :],
                             xvs[NXT - 1][0:DIM, XROWS - 1:XROWS,
                                          1:1 + w].bitcast(F32),
                             start=True, stop=True)
            nc.scalar.activation(vv[0:DIM, PH2 - 1:PH2, 1:1 + w],
                                 hps_a[:].rearrange("p (r c) -> p r c", c=w),
                                 AF.Identity, bias=wt["c1bd"][0:DIM, :])
            hps_b = ps1.tile([DIM, w], F32, tag="vps")
            nc.tensor.matmul(hps_b[:], wt["c1th"][DIM:2 * DIM, :],
                             xvs[0][DIM:2 * DIM, 0:1, 1:1 + w].bitcast(F32),
                             start=True, stop=True)
            nc.scalar.activation(vv[DIM:2 * DIM, 0:1, 1:1 + w],
                                 hps_b[:].rearrange("p (r c) -> p r c", c=w),
                                 AF.Identity, bias=wt["c1bd"][DIM:2 * DIM, :])

            for i in range(ntiles):
                r0 = R * i
                sl = slice(i * NT, (i + 1) * NT)
                xb = xvs[i // TPX]
                rl = R * (i % TPX)

                kps = ps1.tile([2 * DIM, NT], F32, tag="kps")
                for t, (di, dj) in enumerate(TAPS):
                    nc.tensor.matmul(
                        kps[:], wt["kwd"][:, t * 2 * DIM:(t + 1) * 2 * DIM],
                        xb[:, rl + di:rl + di + R, dj:dj + w],
                        start=(t == 0), stop=(t == K2 - 1))
                nc.scalar.activation(KB[:, sl], kps[:], AF.Relu,
                                     bias=wt["kebd"][:],
                                     accum_out=ksums[:, i:i + 1])

                wps = ps1.tile([2 * ATTN, NT], F32, tag="wps")
                nc.tensor.matmul(wps[:], wt["w1wx"][:],
                                 xb[:, rl + 1:rl + 1 + R, 1:1 + w],
                                 start=True, stop=False)
                nc.tensor.matmul(wps[:], wt["w1wk"][:],
                                 KB[:, sl], start=False, stop=True)
                nc.scalar.activation(W1B[:, sl], wps[:], AF.Relu,
                                     bias=wt["w1bd"][:])

                vps = ps1.tile([2 * DIM, NT], F32, tag="vps")
                nc.tensor.matmul(vps[:], wt["c1td"][:],
                                 xb[:, rl + 1:rl + 1 + R, 1:1 + w],
                                 start=True, stop=True)
                nc.scalar.activation(vv[:, r0 + 1:r0 + 1 + R, 1:1 + w],
                                     r3(vps[:]), AF.Identity, bias=wt["c1bd"][:])

                zpa = ps1.tile([EMB, NT], F32, tag="zps")
                nc.tensor.matmul(zpa[:], wt["e3272d"][0:ATTN, :],
                                 W1B[0:ATTN, sl], start=True, stop=True)
                nc.vector.bn_stats(stats6[:, 12 * i:12 * i + 6], zpa[:])
                zpb = ps1.tile([EMB, NT], F32, tag="zps")
                nc.tensor.matmul(zpb[:], wt["e3272d"][ATTN:2 * ATTN, :],
                                 W1B[ATTN:2 * ATTN, sl], start=True, stop=True)
                nc.vector.bn_stats(stats6[:, 12 * i + 6:12 * i + 12], zpb[:])

        with tc.tile_pool(name="ps2", bufs=2, space="PSUM") as ps2:
            # ---------------- GN stats -> per-tap affine ----------------
            mv = wp.tile([EMB, 2], F32)
            nc.vector.bn_aggr(mv[:], stats6[:])
            st2 = wp.tile([EMB, 2], F32)
            nc.vector.tensor_copy(st2[:, 0:1], mv[:, 0:1])
            sqm = wp.tile([EMB, 1], F32)
            nc.vector.tensor_mul(sqm[:], mv[:, 0:1], mv[:, 0:1])
            nc.vector.tensor_add(st2[:, 1:2], mv[:, 1:2], sqm[:])

            gps = ps2.tile([G, 3], F32, tag="sm")
            nc.tensor.matmul(gps[:, 0:2], wt["g8"][:], st2[:],
                             start=True, stop=True, skip_group_check=True)
            nc.tensor.matmul(gps[:, 2:3], wt["gb8"][:], st2[:, 0:1],
                             start=True, stop=True, skip_group_check=True)
            s3 = wp.tile([G, 3], F32)
            nc.scalar.activation(s3[:], gps[:], AF.Copy)

            mg = wp.tile([G, 1], F32)
            nc.vector.tensor_add(mg[:], s3[:, 0:1], wt["mbg"][:])
            t1 = wp.tile([G, 1], F32)
            nc.vector.tensor_scalar_mul(t1[:], s3[:, 2:3], 2.0)
            t2 = wp.tile([G, 1], F32)
            nc.vector.tensor_add(t2[:], s3[:, 1:2], t1[:])
            ex2 = wp.tile([G, 1], F32)
            nc.vector.tensor_add(ex2[:], t2[:], wt["b2sq"][:])
            mg2 = wp.tile([G, 1], F32)
            nc.vector.tensor_mul(mg2[:], mg[:], mg[:])
            varg = wp.tile([G, 1], F32)
            nc.vector.tensor_sub(varg[:], ex2[:], mg2[:])
            stdg = wp.tile([G, 1], F32)
            nc.scalar.activation(stdg[:], varg[:], AF.Sqrt, bias=epst[:])
            rstd = wp.tile([G, 1], F32)
            nc.vector.reciprocal(rstd[:], stdg[:])

            mrr = wp.tile([G, 2], F32)
            nc.vector.tensor_copy(mrr[:, 0:1], mg[:])
            nc.vector.tensor_copy(mrr[:, 1:2], rstd[:])
            mps = ps2.tile([EMB, 2], F32, tag="sm")
            nc.tensor.matmul(mps[:], wt["x8to72"][:], mrr[:],
                             start=True, stop=True)
            mr72 = wp.tile([EMB, 2], F32)
            nc.scalar.activation(mr72[:], mps[:], AF.Copy)

            al72 = wp.tile([EMB, 1], F32)
            nc.vector.tensor_mul(al72[:], wt["gng"][:], mr72[:, 1:2])
            dcol = wp.tile([EMB, 1], F32)
            nc.vector.tensor_sub(dcol[:], wt["b2c"][:], mr72[:, 0:1])
            tb = wp.tile([EMB, 1], F32)
            nc.vector.tensor_mul(tb[:], al72[:], dcol[:])
            be72 = wp.tile([EMB, 1], F32)
            nc.vector.tensor_add(be72[:], tb[:], wt["gnb"][:])

            alE = wp.tile([EMB, ATTN], F32)
            nc.vector.tensor_scalar_mul(alE[:], wt["e72"][:], al72[:])

            lpsA = ps2.tile([ATTN, 8 * DIM], F32, tag="qps", bufs=2)
            nc.tensor.matmul(lpsA[:], alE[:], wt["sel"][:, 0:8 * DIM],
                             start=True, stop=True)
            nc.scalar.activation(
                LT2v[0:ATTN, 0:8, 0:DIM],
                lpsA.rearrange("p (t c) -> p t c", c=DIM), AF.Copy)
            lpsB = ps2.tile([ATTN, DIM], F32, tag="sm", name="lpsB")
            nc.tensor.matmul(lpsB[:], alE[:], wt["sel"][:, 8 * DIM:9 * DIM],
                             start=True, stop=True)
            nc.scalar.activation(LT2v[0:ATTN, 8:9, 0:DIM], lpsB[:].unsqueeze(1),
                                 AF.Copy)
            nc.sync.dma_start(out=LT2v[ATTN:2 * ATTN, :, DIM:2 * DIM],
                              in_=LT2v[0:ATTN, :, 0:DIM])
            nc.sync.dma_start(out=LT2v[ATTN:2 * ATTN, :, 0:DIM],
                              in_=LT2v[0:ATTN, :, DIM:2 * DIM])
            bps = ps2.tile([DIM, K2], F32, tag="sm")
            for t in range(K2):
                nc.tensor.matmul(bps[:, t:t + 1],
                                 wt["sel"][:, t * DIM:(t + 1) * DIM],
                                 be72[:], start=True, stop=True,
                                 skip_group_check=True)
            nc.scalar.activation(BT2[0:DIM, :], bps[:], AF.Copy)
            nc.sync.dma_start(out=BT2[DIM:2 * DIM, :], in_=BT2[0:DIM, :])

            # ---------------- phase 2: dynamic aggregation + silu ----------------
            # taps whose PSUM q is staged to SBUF bf16 by the (otherwise idle)
            # Act engine so the DVE multiply runs in 2x bf16 mode
            ACT_TAPS = (0, 1, 2, 4, 6, 8)
            for i in range(ntiles):
                r0 = R * i
                sl = slice(i * NT, (i + 1) * NT)
                pts = []
                for t, (di, dj) in enumerate(TAPS):
                    qps = ps2.tile([2 * DIM, NT], F32, tag="qps",
                                   name=f"q{i}_{t}")
                    nc.tensor.matmul(qps[:],
                                     LT2[:, t * 2 * DIM:(t + 1) * 2 * DIM],
                                     W1B[:, sl], start=True, stop=True)
                    vop = vv[:, r0 + di:r0 + di + R, dj:dj + w]
                    # DVE-pair taps share tags pa/pb, Pool-pair taps pc/pd
                    # (deeper bufs: Pool drains slowly), the last tap pe
                    ptag = {0: "pa", 1: "pb", 2: "pc", 3: "pd", 4: "pa",
                            5: "pb", 6: "pc", 7: "pd", 8: "pe"}[t]
                    pbufs = 3
                    pt = wkp.tile([2 * DIM, NT], BF16, tag=ptag, bufs=pbufs,
                                  name=f"pt{i}_{t}")
                    if t in ACT_TAPS:
                        qb = wkp.tile([2 * DIM, NT], BF16,
                                      tag={0: "qa", 4: "qa", 8: "qa", 1: "qd", 2: "qc", 6: "qc"}[t],
                                      bufs=3, name=f"qb{i}_{t}")
                        nc.scalar.activation(qb[:], qps[:], AF.Identity,
                                             bias=BT2[:, t:t + 1])
                        if t == 2:
                            # tap 2 feeds Pool's add chain: multiply there too
                            nc.gpsimd.tensor_mul(r3(pt[:]), r3(qb[:]), vop)
                        else:
                            nc.vector.tensor_mul(r3(pt[:]), r3(qb[:]), vop)
                    else:
                        nc.vector.scalar_tensor_tensor(
                            r3(pt[:]), r3(qps[:]),
                            BT2[:, t:t + 1], vop, op0=OP.add, op1=OP.mult)
                    pts.append(pt)
                # pairwise add tree: two pair-sums on Pool, the rest on DVE
                # two independent same-engine chains (DVE and Pool), one
                # cross-engine join at the end -- avoids per-level ping-pong
                d1 = wkp.tile([2 * DIM, NT], BF16, tag="s1", bufs=2,
                              name=f"d1_{i}")
                nc.vector.tensor_add(d1[:], pts[0][:], pts[1][:])
                c1 = wkp.tile([2 * DIM, NT], BF16, tag="s2", bufs=2,
                              name=f"c1_{i}")
                nc.gpsimd.tensor_add(c1[:], pts[2][:], pts[3][:])
                d2 = wkp.tile([2 * DIM, NT], BF16, tag="s3", bufs=2,
                              name=f"d2_{i}")
                nc.vector.tensor_add(d2[:], d1[:], pts[4][:])
                c2 = wkp.tile([2 * DIM, NT], BF16, tag="s4", bufs=2,
                              name=f"c2_{i}")
                nc.gpsimd.tensor_add(c2[:], c1[:], pts[6][:])
                d3 = wkp.tile([2 * DIM, NT], BF16, tag="u1", bufs=2,
                              name=f"d3_{i}")
                nc.vector.tensor_add(d3[:], d2[:], pts[5][:])
                c3 = wkp.tile([2 * DIM, NT], BF16, tag="u2", bufs=2,
                              name=f"c3_{i}")
                nc.gpsimd.tensor_add(c3[:], c2[:], pts[7][:])
                d4 = wkp.tile([2 * DIM, NT], BF16, tag="u3", bufs=2,
                              name=f"d4_{i}")
                nc.vector.tensor_add(d4[:], d3[:], pts[8][:])
                acc = wkp.tile([2 * DIM, NT], BF16, tag="acc", bufs=2,
                               name=f"acc{i}")
                nc.vector.tensor_add(acc[:], d4[:], c3[:])
                nc.scalar.activation(VAGG[:, sl], acc[:], AF.Silu,
                                     bias=wt["bnbd"][:],
                                     accum_out=vsums[:, i:i + 1])

            # ---------------- SE gating ----------------
            ks = wp.tile([2 * DIM, 1], F32)
            nc.vector.reduce_sum(ks[:], ksums[:], axis=mybir.AxisListType.X)
            vs = wp.tile([2 * DIM, 1], F32)
            nc.vector.reduce_sum(vs[:], vsums[:], axis=mybir.AxisListType.X)
            g0 = wp.tile([2 * DIM, 1], F32)
            nc.vector.tensor_add(g0[:], ks[:], vs[:])
            gp0 = ps2.tile([DIM, 1], F32, tag="sm")
            nc.tensor.matmul(gp0[:], wt["fold2"][:], g0[:], start=True, stop=True)
            gap = wp.tile([DIM, 1], F32)
            nc.scalar.activation(gap[:], gp0[:], AF.Copy, scale=1.0 / HW)

            sps1 = ps2.tile([ATTN, 1], F32, tag="sm")
            nc.tensor.matmul(sps1[:], wt["sew1"][:], gap[:], start=True, stop=True)
            a1se = wp.tile([ATTN, 1], F32)
            nc.scalar.activation(a1se[:], sps1[:], AF.Relu, bias=wt["seb1"][:])
            sps2 = ps2.tile([RADIX * DIM, 1], F32, tag="sm")
            nc.tensor.matmul(sps2[:], wt["sew2"][:], a1se[:], start=True, stop=True)
            av = wp.tile([RADIX * DIM, 1], F32)
            nc.scalar.activation(av[:], sps2[:], AF.Identity, bias=wt["seb2"][:])
            sps3 = ps2.tile([DIM, 1], F32, tag="sm")
            nc.tensor.matmul(sps3[:], wt["dif"][:], av[:], start=True, stop=True)
            a0 = wp.tile([DIM, 1], F32)
            nc.scalar.activation(a0[:], sps3[:], AF.Sigmoid, bias=zbias[0:DIM, :])
            a1c = wp.tile([DIM, 1], F32)
            nc.vector.tensor_scalar(a1c[:], a0[:], -1.0, 1.0,
                                    op0=OP.mult, op1=OP.add)

            a0row = wp.tile([1, 2 * DIM], F32)
            nc.sync.dma_start(out=a0row[0:1, 0:DIM], in_=a0[:])
            nc.sync.dma_start(out=a0row[0:1, DIM:2 * DIM], in_=a0[:])
            a1row = wp.tile([1, 2 * DIM], F32)
            nc.sync.dma_start(out=a1row[0:1, 0:DIM], in_=a1c[:])
            nc.sync.dma_start(out=a1row[0:1, DIM:2 * DIM], in_=a1c[:])
            rp0 = ps2.tile([2 * DIM, 2 * DIM], F32, tag="sm")
            nc.tensor.matmul(rp0[:], wt["ones1"][:], a0row[:], start=True, stop=True)
            nc.vector.tensor_mul(diag0[:], wt["i128"][:], rp0[:])
            rp1 = ps2.tile([2 * DIM, 2 * DIM], F32, tag="sm")
            nc.tensor.matmul(rp1[:], wt["ones1"][:], a1row[:], start=True, stop=True)
            nc.vector.tensor_mul(diag1[:], wt["i128"][:], rp1[:])

            # ---------------- phase 3: blend + store ----------------
            BT = 2
            for bi in range(ntiles // BT):
                ops_ = ps2.tile([2 * DIM, BT * NT], F32, tag="bps",
                                bufs=2, name=f"ob{bi}")
                for j in range(BT):
                    i = bi * BT + j
                    sl = slice(i * NT, (i + 1) * NT)
                    psl = slice(j * NT, (j + 1) * NT)
                    nc.tensor.matmul(ops_[:, psl], diag0[:], VAGG[:, sl],
                                     start=True, stop=False)
                    nc.tensor.matmul(ops_[:, psl], diag1[:], KB[:, sl],
                                     start=False, stop=True)
                r0 = R * BT * bi
                ost = wkp.tile([2 * DIM, BT * NT], F32, tag="ost", bufs=2,
                               name=f"os{bi}")
                nc.vector.tensor_copy(ost[:], ops_[:])
                ov = ost.rearrange("p (r c) -> p r c", c=w)
                nc.sync.dma_start(out=od[:, r0:r0 + R * BT, :],
                                  in_=ov[0:DIM, :, :])
                nc.sync.dma_start(out=od[:, HH + r0:HH + r0 + R * BT, :],
                                  in_=ov[DIM:2 * DIM, :, :])

    nc.compile()
    return nc


# ---------------------------------------------------------------- entry point

_NC_CACHE = {}


def _get_nc(h, w):
    key = (h, w)
    if key not in _NC_CACHE:
        _NC_CACHE[key] = build_nc(h, w)
    return _NC_CACHE[key]


def make_in_maps(inputs, h=H, w=W):
    wts = prep_weights(inputs)
    x = np.asarray(inputs["x"], np.float32)
    maps = []
    for b in range(x.shape[0]):
        m = dict(wts)
        m["x"] = pack_x(x[b], h, w)
        maps.append(m)
    return maps


def kernel(**inputs):
    from concourse.bass_utils import run_bass_kernel_spmd
    x = np.asarray(inputs["x"], np.float32)
    b, c, h, w = x.shape
    assert b == N_CORES and c == DIM
    nc = _get_nc(h, w)
    in_maps = make_in_maps(inputs, h, w)
    res = run_bass_kernel_spmd(nc, in_maps, list(range(N_CORES)))
    out = np.stack([res.results[i]["out"] for i in range(N_CORES)], axis=0)
    return out.astype(np.float32)



# revision 13
# speedup vs baseline: 1.0021x; 1.0021x over previous
"""Trainium2 Bass kernel for nn_CoTLayer (CoT attention layer).

Self-contained: takes FULL inputs (as produced by setup_inputs()), shards
batch across 8 NeuronCores (data-parallel, weights replicated), runs a
Bass/Tile kernel per core, returns the FULL output.

Per-core layout ("dual-half" packing, one batch image per core):
  - partitions p = c + 64*half: channels 0..63 of the TOP image half on
    partitions 0..63, same channels of the BOTTOM half on 64..127 -> every
    engine pass runs 128 partitions wide at half the free-dim length.
  - x and v stored zero-padded (128, PH2, PW) so 3x3 taps are strided AP
    reads; one halo row per half is computed redundantly.
  - k, w1, vAgg stored dense; matmuls stream interior-only 3D APs, so no
    pad garbage pollutes GN statistics or the global average pool.
"""

import numpy as np
import ml_dtypes
from contextlib import ExitStack

import concourse.bass as bass
import concourse.tile as tile
from concourse import bacc, mybir

F32 = mybir.dt.float32
F32R = mybir.dt.float32r
BF16 = mybir.dt.bfloat16
AF = mybir.ActivationFunctionType
OP = mybir.AluOpType


# float32r (TF32-like) matmuls run 4x faster than fp32 on the PE at
# free-size >= 256. The BIR verifier requires every producer of an f32r
# matmul operand to write f32r (DMA/Act/DVE all qualify; Memset does not),
# so the big matmul-feeding tiles are declared f32r and their DMAs bitcast.
R_TILES = {"kwd", "w1wx", "c1td", "e3272d"}
BF_TILES = {"w1wk"}

B, DIM, H, W = 8, 64, 128, 128
KS, K2, SP, RADIX = 3, 9, 8, 2
ATTN = 32
EMB = 72
G = DIM // SP          # 8 groups
GS = EMB // G          # 9 channels per group
EPS = 1e-5
N_CORES = 8

TAPS = [(ki, kj) for ki in range(KS) for kj in range(KS)]  # t = ki*3+kj


# ---------------------------------------------------------------- host prep

def _blockdiag2(a):
    """(k, m) -> (2k, 2m) with a on both diagonal blocks."""
    k, m = a.shape
    out = np.zeros((2 * k, 2 * m), np.float32)
    out[:k, :m] = a
    out[k:, m:] = a
    return out


def prep_weights(inp):
    """Fold BN scales into weights host-side; build matmul-ready tensors."""
    f = np.float32
    sc = 1.0 / np.sqrt(f(1.0 + EPS))

    ke_w = np.asarray(inp["ke_w"], f)            # (64, 16, 3, 3)
    sck = np.asarray(inp["ke_g"], f) * sc        # (64,)
    kwd = np.zeros((2 * DIM, K2 * 2 * DIM), f)   # dual lhsT per tap
    for t, (ki, kj) in enumerate(TAPS):
        blk = np.zeros((DIM, DIM), f)            # (in, out)
        for o in range(DIM):
            g = o // 16
            blk[16 * g:16 * g + 16, o] = ke_w[o, :, ki, kj] * sck[o]
        kwd[:, t * 2 * DIM:(t + 1) * 2 * DIM] = _blockdiag2(blk)

    sc1 = np.asarray(inp["em_g1"], f) * sc       # (32,)
    em_w1 = np.asarray(inp["em_w1"], f)          # (32, 128)
    w1w = (em_w1 * sc1[:, None]).T.copy()        # (128, 32): rows 0-63 x part
    w1wx = _blockdiag2(w1w[:DIM])                # (128, 64)
    w1wk = _blockdiag2(w1w[DIM:])                # (128, 64)

    scc = np.asarray(inp["c1_g"], f) * sc
    c1t = (np.asarray(inp["c1_w"], f) * scc[:, None]).T.copy()   # (64, 64)
    c1td = _blockdiag2(c1t)                      # (128, 128)
    c1th = np.vstack([c1t, c1t])                 # (128, 64) for halo rows

    em_w2 = np.asarray(inp["em_w2"], f)          # (72, 32)
    e3272d = np.vstack([em_w2.T, em_w2.T]).copy()  # (64, 72)
    e72 = em_w2.copy()                           # (72, 32)

    sc_bn = np.asarray(inp["bn_g"], f) * sc      # (64,) silu-BN scale
    sel = np.zeros((EMB, K2 * DIM), f)
    for t in range(K2):
        for c in range(DIM):
            sel[(c % SP) * GS + t, t * DIM + c] = sc_bn[c]

    b2 = np.asarray(inp["em_b2"], f)             # (72,)
    g8 = np.zeros((EMB, G), f)
    gb8 = np.zeros((EMB, G), f)
    x8to72 = np.zeros((G, EMB), f)
    for e in range(EMB):
        g8[e, e // GS] = 1.0 / GS
        gb8[e, e // GS] = b2[e] / GS
        x8to72[e // GS, e] = 1.0
    mbg = np.array([b2[9 * g:9 * g + 9].mean() for g in range(G)], f)[:, None]
    b2sq = np.array([(b2[9 * g:9 * g + 9] ** 2).mean() for g in range(G)], f)[:, None]

    scse = np.asarray(inp["se_g"], f) * sc
    sew1 = (np.asarray(inp["se_w1"], f) * scse[:, None]).T.copy()   # (64, 32)
    seb1 = (np.asarray(inp["se_b1"], f) * scse + np.asarray(inp["se_bb"], f))[:, None]
    sew2 = np.asarray(inp["se_w2"], f).T.copy()                      # (32, 128)

    dif = np.zeros((RADIX * DIM, DIM), f)
    for m in range(DIM):
        dif[2 * m, m] = 1.0
        dif[2 * m + 1, m] = -1.0

    fold2 = np.vstack([np.eye(DIM, dtype=f), np.eye(DIM, dtype=f)])  # (128, 64)

    def dup(v):   # (64,1)->(128,1)
        return np.vstack([v, v]).astype(f)

    keb = np.asarray(inp["ke_b"], f)[:, None]
    c1b = np.asarray(inp["c1_b"], f)[:, None]
    w1b = np.asarray(inp["em_b1"], f)[:, None]
    bnb = np.asarray(inp["bn_b"], f)[:, None]

    return {
        "kwd": kwd,
        "kebd": dup(keb),
        "w1wx": w1wx,
        "w1wk": w1wk.astype(ml_dtypes.bfloat16),
        "w1bd": np.vstack([w1b, w1b]).astype(f),     # (64, 1)
        "c1td": c1td,
        "c1th": c1th,
        "c1bd": dup(c1b),
        "e3272d": e3272d,
        "e72": e72,
        "sel": sel,
        "g8": g8,
        "gb8": gb8,
        "x8to72": x8to72,
        "mbg": mbg,
        "b2sq": b2sq,
        "b2c": b2[:, None].copy(),
        "gng": np.asarray(inp["em_gn_g"], f)[:, None],
        "gnb": np.asarray(inp["em_gn_b"], f)[:, None],
        "bnbd": dup(bnb),
        "fold2": fold2,
        "sew1": sew1,
        "seb1": seb1,
        "sew2": sew2,
        "seb2": np.asarray(inp["se_b2"], f)[:, None],
        "dif": dif,
        "i128": np.eye(2 * DIM, dtype=f),
        "ones1": np.ones((1, 2 * DIM), f),
    }


def pack_x(xb, h, w):
    """(64, h, w) -> dual-half padded (128, HH+2, w+4)."""
    HH = h // 2
    PH2, PW = HH + 2, w + 4
    xp = np.zeros((2 * DIM, PH2, PW), np.float32)
    xb = np.asarray(xb, np.float32)
    # half A: buffer row r = image row r-1 (rows 1..HH+1 = image 0..HH)
    xp[:DIM, 1:PH2, 1:w + 1] = xb[:, 0:HH + 1, :]
    # half B: buffer row r = image row HH-1+r (rows 0..HH = image HH-1..h-1)
    xp[DIM:, 0:PH2 - 1, 1:w + 1] = xb[:, HH - 1:h, :]
    return xp


# ---------------------------------------------------------------- device code

def build_nc(h=H, w=W):
    """Build the per-core Bass program (parametric spatial size for testing)."""
    HH = h // 2
    PH2, PW = HH + 2, w + 4
    R = max(1, min(HH, 512 // w))     # image rows per tile (per half)
    assert HH % R == 0
    NT = R * w                        # matmul free size per tile (<=512)
    ntiles = HH // R
    HW = h * w
    HW2 = HH * w

    nc = bacc.Bacc("TRN2", target_bir_lowering=False, debug=False,
                   num_devices=N_CORES)

    dp = nc.declare_dram_parameter
    xd = dp("x", [2 * DIM, PH2, PW], F32, isOutput=False)
    names = {
        "kwd": [2 * DIM, K2 * 2 * DIM], "kebd": [2 * DIM, 1],
        "w1wx": [2 * DIM, 2 * ATTN], "w1wk": [2 * DIM, 2 * ATTN],
        "w1bd": [2 * ATTN, 1], "c1td": [2 * DIM, 2 * DIM],
        "c1th": [2 * DIM, DIM], "c1bd": [2 * DIM, 1],
        "e3272d": [2 * ATTN, EMB], "e72": [EMB, ATTN], "sel": [EMB, K2 * DIM],
        "g8": [EMB, G], "gb8": [EMB, G], "x8to72": [G, EMB],
        "mbg": [G, 1], "b2sq": [G, 1], "b2c": [EMB, 1], "gng": [EMB, 1],
        "gnb": [EMB, 1], "bnbd": [2 * DIM, 1], "fold2": [2 * DIM, DIM],
        "sew1": [DIM, ATTN], "seb1": [ATTN, 1],
        "sew2": [ATTN, RADIX * DIM], "seb2": [RADIX * DIM, 1],
        "dif": [RADIX * DIM, DIM], "i128": [2 * DIM, 2 * DIM],
        "ones1": [1, 2 * DIM],
    }
    wd = {k: dp(k, s, BF16 if k in BF_TILES else F32, isOutput=False)
          for k, s in names.items()}
    od = dp("out", [DIM, h, w], F32, isOutput=True)

    with tile.TileContext(nc) as tc, ExitStack() as ctx:
        wp = ctx.enter_context(tc.tile_pool(name="wp", bufs=1))
        bigp = ctx.enter_context(tc.tile_pool(name="bigp", bufs=1))
        wkp = ctx.enter_context(tc.tile_pool(name="wkp", bufs=2))

        # --- persistent SBUF ---
        # x is split into NXT row-band tiles so phase-1 compute on band 0 can
        # start as soon as its DMA lands instead of waiting for the full load.
        TPX = min(4, ntiles)              # compute tiles per x band
        assert ntiles % TPX == 0
        NXT = ntiles // TPX
        XROWS = TPX * R + 2               # band rows incl. 3x3 halo
        XTs = [bigp.tile([128, XROWS * PW], F32R, name=f"xt{j}")
               for j in range(NXT)]
        xvs = [t_.rearrange("p (r c) -> p r c", c=PW) for t_ in XTs]
        VPB = bigp.tile([128, PH2 * PW], BF16)        # padded v
        KB = bigp.tile([128, HW2], BF16)              # dense k
        W1B = bigp.tile([2 * ATTN, HW2], F32R)        # dense w1 (A rows 0-31)
        VAGG = bigp.tile([128, HW2], F32R)            # silu'd aggregation
        vv = VPB.rearrange("p (r c) -> p r c", c=PW)

        def _wdt(k):
            if k in R_TILES:
                return F32R
            return BF16 if k in BF_TILES else F32
        wt = {k: wp.tile(list(v.shape), _wdt(k), name=f"w_{k}")
              for k, v in wd.items()}
        zbias = wp.tile([128, 1], F32)
        nc.vector.memset(zbias[:], 0.0)
        epst = wp.tile([G, 1], F32)
        nc.vector.memset(epst[:], EPS)
        stats6 = wp.tile([EMB, 12 * ntiles], F32)
        ksums = wp.tile([2 * DIM, ntiles], F32)
        vsums = wp.tile([2 * DIM, ntiles], F32)
        LT2 = wp.tile([2 * ATTN, K2 * 2 * DIM], F32R)
        LT2v = LT2.rearrange("p (t c) -> p t c", c=2 * DIM)
        BT2 = wp.tile([2 * DIM, K2], F32)
        diag0 = wp.tile([2 * DIM, 2 * DIM], F32R)
        diag1 = wp.tile([2 * DIM, 2 * DIM], BF16)

        # --- load x band 0, weights, then remaining bands (in compute order)
        def load_band(j):
            base = TPX * R * j
            ch = max(1, XROWS // 3)
            for r in range(0, XROWS, ch):
                r2 = min(XROWS, r + ch)
                nc.sync.dma_start(
                    out=xvs[j][:, r:r2, :],
                    in_=xd[:, base + r:base + r2, :].bitcast(F32R))
        first = ["kwd", "c1th", "c1td", "w1wx", "w1wk", "kebd", "c1bd",
                 "w1bd", "e3272d"]
        def load_w(k):
            src = wd[k][:].bitcast(F32R) if k in R_TILES else wd[k][:]
            nc.sync.dma_start(out=wt[k][:], in_=src)
        for k in first:
            load_w(k)
        load_band(0)
        for k in wt:
            if k not in first:
                load_w(k)
        for j in range(1, NXT):
            load_band(j)
        # zero the off-diagonal blocks of LT2 once (Act writes f32r; a plain
        # memset would not satisfy the f32r-producer rule)
        nc.scalar.activation(
            LT2v[0:ATTN, :, DIM:2 * DIM],
            wt["sel"][0:ATTN, :].rearrange("p (t c) -> p t c", c=DIM),
            AF.Copy, scale=0.0)
        nc.gpsimd.memset(vv[:, 0:1, :], 0.0)
        nc.gpsimd.memset(vv[:, PH2 - 1:PH2, :], 0.0)
        nc.gpsimd.memset(vv[:, 1:PH2 - 1, 0:1], 0.0)
        nc.gpsimd.memset(vv[:, 1:PH2 - 1, w + 1:PW], 0.0)

        def r3(ap2d):  # dense (p, NT) -> (p, R, w)
            return ap2d.rearrange("p (r c) -> p r c", c=w)

        # ---------------- phase 1: k, w1, v, GN stats ----------------
        with tc.tile_pool(name="ps1", bufs=2, space="PSUM") as ps1:
            # v halo rows (one per half), computed redundantly
            hps_a = ps1.tile([DIM, w], F32, tag="vps")
            nc.tensor.matmul(hps_a[:], wt["c1th"][0:DIM, :],
                             xvs[NXT - 1][0:DIM, XROWS - 1:XROWS,
                                          1:1 + w].bitcast(F32),
                             start=True, stop=True)
            nc.scalar.activation(vv[0:DIM, PH2 - 1:PH2, 1:1 + w],
                                 hps_a[:].rearrange("p (r c) -> p r c", c=w),
                                 AF.Identity, bias=wt["c1bd"][0:DIM, :])
            hps_b = ps1.tile([DIM, w], F32, tag="vps")
            nc.tensor.matmul(hps_b[:], wt["c1th"][DIM:2 * DIM, :],
                             xvs[0][DIM:2 * DIM, 0:1, 1:1 + w].bitcast(F32),
                             start=True, stop=True)
            nc.scalar.activation(vv[DIM:2 * DIM, 0:1, 1:1 + w],
                                 hps_b[:].rearrange("p (r c) -> p r c", c=w),
                                 AF.Identity, bias=wt["c1bd"][DIM:2 * DIM, :])

            for i in range(ntiles):
                r0 = R * i
                sl = slice(i * NT, (i + 1) * NT)
                xb = xvs[i // TPX]
                rl = R * (i % TPX)

                kps = ps1.tile([2 * DIM, NT], F32, tag="kps")
                for t, (di, dj) in enumerate(TAPS):
                    nc.tensor.matmul(
                        kps[:], wt["kwd"][:, t * 2 * DIM:(t + 1) * 2 * DIM],
                        xb[:, rl + di:rl + di + R, dj:dj + w],
                        start=(t == 0), stop=(t == K2 - 1))
                nc.scalar.activation(KB[:, sl], kps[:], AF.Relu,
                                     bias=wt["kebd"][:],
                                     accum_out=ksums[:, i:i + 1])

                wps = ps1.tile([2 * ATTN, NT], F32, tag="wps")
                nc.tensor.matmul(wps[:], wt["w1wx"][:],
                                 xb[:, rl + 1:rl + 1 + R, 1:1 + w],
                                 start=True, stop=False)
                nc.tensor.matmul(wps[:], wt["w1wk"][:],
                                 KB[:, sl], start=False, stop=True)
                nc.scalar.activation(W1B[:, sl], wps[:], AF.Relu,
                                     bias=wt["w1bd"][:])

                vps = ps1.tile([2 * DIM, NT], F32, tag="vps")
                nc.tensor.matmul(vps[:], wt["c1td"][:],
                                 xb[:, rl + 1:rl + 1 + R, 1:1 + w],
                                 start=True, stop=True)
                nc.scalar.activation(vv[:, r0 + 1:r0 + 1 + R, 1:1 + w],
                                     r3(vps[:]), AF.Identity, bias=wt["c1bd"][:])

                zpa = ps1.tile([EMB, NT], F32, tag="zps")
                nc.tensor.matmul(zpa[:], wt["e3272d"][0:ATTN, :],
                                 W1B[0:ATTN, sl], start=True, stop=True)
                nc.vector.bn_stats(stats6[:, 12 * i:12 * i + 6], zpa[:])
                zpb = ps1.tile([EMB, NT], F32, tag="zps")
                nc.tensor.matmul(zpb[:], wt["e3272d"][ATTN:2 * ATTN, :],
                                 W1B[ATTN:2 * ATTN, sl], start=True, stop=True)
                nc.vector.bn_stats(stats6[:, 12 * i + 6:12 * i + 12], zpb[:])

        with tc.tile_pool(name="ps2", bufs=2, space="PSUM") as ps2:
            # ---------------- GN stats -> per-tap affine ----------------
            mv = wp.tile([EMB, 2], F32)
            nc.vector.bn_aggr(mv[:], stats6[:])
            st2 = wp.tile([EMB, 2], F32)
            nc.vector.tensor_copy(st2[:, 0:1], mv[:, 0:1])
            sqm = wp.tile([EMB, 1], F32)
            nc.vector.tensor_mul(sqm[:], mv[:, 0:1], mv[:, 0:1])
            nc.vector.tensor_add(st2[:, 1:2], mv[:, 1:2], sqm[:])

            gps = ps2.tile([G, 3], F32, tag="sm")
            nc.tensor.matmul(gps[:, 0:2], wt["g8"][:], st2[:],
                             start=True, stop=True, skip_group_check=True)
            nc.tensor.matmul(gps[:, 2:3], wt["gb8"][:], st2[:, 0:1],
                             start=True, stop=True, skip_group_check=True)
            s3 = wp.tile([G, 3], F32)
            nc.scalar.activation(s3[:], gps[:], AF.Copy)

            mg = wp.tile([G, 1], F32)
            nc.vector.tensor_add(mg[:], s3[:, 0:1], wt["mbg"][:])
            t1 = wp.tile([G, 1], F32)
            nc.vector.tensor_scalar_mul(t1[:], s3[:, 2:3], 2.0)
            t2 = wp.tile([G, 1], F32)
            nc.vector.tensor_add(t2[:], s3[:, 1:2], t1[:])
            ex2 = wp.tile([G, 1], F32)
            nc.vector.tensor_add(ex2[:], t2[:], wt["b2sq"][:])
            mg2 = wp.tile([G, 1], F32)
            nc.vector.tensor_mul(mg2[:], mg[:], mg[:])
            varg = wp.tile([G, 1], F32)
            nc.vector.tensor_sub(varg[:], ex2[:], mg2[:])
            stdg = wp.tile([G, 1], F32)
            nc.scalar.activation(stdg[:], varg[:], AF.Sqrt, bias=epst[:])
            rstd = wp.tile([G, 1], F32)
            nc.vector.reciprocal(rstd[:], stdg[:])

            mrr = wp.tile([G, 2], F32)
            nc.vector.tensor_copy(mrr[:, 0:1], mg[:])
            nc.vector.tensor_copy(mrr[:, 1:2], rstd[:])
            mps = ps2.tile([EMB, 2], F32, tag="sm")
            nc.tensor.matmul(mps[:], wt["x8to72"][:], mrr[:],
                             start=True, stop=True)
            mr72 = wp.tile([EMB, 2], F32)
            nc.scalar.activation(mr72[:], mps[:], AF.Copy)

            al72 = wp.tile([EMB, 1], F32)
            nc.vector.tensor_mul(al72[:], wt["gng"][:], mr72[:, 1:2])
            dcol = wp.tile([EMB, 1], F32)
            nc.vector.tensor_sub(dcol[:], wt["b2c"][:], mr72[:, 0:1])
            tb = wp.tile([EMB, 1], F32)
            nc.vector.tensor_mul(tb[:], al72[:], dcol[:])
            be72 = wp.tile([EMB, 1], F32)
            nc.vector.tensor_add(be72[:], tb[:], wt["gnb"][:])

            alE = wp.tile([EMB, ATTN], F32)
            nc.vector.tensor_scalar_mul(alE[:], wt["e72"][:], al72[:])

            lpsA = ps2.tile([ATTN, 8 * DIM], F32, tag="qps", bufs=2)
            nc.tensor.matmul(lpsA[:], alE[:], wt["sel"][:, 0:8 * DIM],
                             start=True, stop=True)
            nc.scalar.activation(
                LT2v[0:ATTN, 0:8, 0:DIM],
                lpsA.rearrange("p (t c) -> p t c", c=DIM), AF.Copy)
            lpsB = ps2.tile([ATTN, DIM], F32, tag="sm", name="lpsB")
            nc.tensor.matmul(lpsB[:], alE[:], wt["sel"][:, 8 * DIM:9 * DIM],
                             start=True, stop=True)
            nc.scalar.activation(LT2v[0:ATTN, 8:9, 0:DIM], lpsB[:].unsqueeze(1),
                                 AF.Copy)
            nc.sync.dma_start(out=LT2v[ATTN:2 * ATTN, :, DIM:2 * DIM],
                              in_=LT2v[0:ATTN, :, 0:DIM])
            nc.sync.dma_start(out=LT2v[ATTN:2 * ATTN, :, 0:DIM],
                              in_=LT2v[0:ATTN, :, DIM:2 * DIM])
            bps = ps2.tile([DIM, K2], F32, tag="sm")
            for t in range(K2):
                nc.tensor.matmul(bps[:, t:t + 1],
                                 wt["sel"][:, t * DIM:(t + 1) * DIM],
                                 be72[:], start=True, stop=True,
                                 skip_group_check=True)
            nc.scalar.activation(BT2[0:DIM, :], bps[:], AF.Copy)
            nc.sync.dma_start(out=BT2[DIM:2 * DIM, :], in_=BT2[0:DIM, :])

            # ---------------- phase 2: dynamic aggregation + silu ----------------
            # taps whose PSUM q is staged to SBUF bf16 by the (otherwise idle)
            # Act engine so the DVE multiply runs in 2x bf16 mode
            ACT_TAPS = (0, 1, 2, 4, 6, 8)
            prev_acc = None
            for i in range(ntiles):
                r0 = R * i
                sl = slice(i * NT, (i + 1) * NT)
                pts = []
                for t, (di, dj) in enumerate(TAPS):
                    qps = ps2.tile([2 * DIM, NT], F32, tag="qps",
                                   name=f"q{i}_{t}")
                    nc.tensor.matmul(qps[:],
                                     LT2[:, t * 2 * DIM:(t + 1) * 2 * DIM],
                                     W1B[:, sl], start=True, stop=True)
                    vop = vv[:, r0 + di:r0 + di + R, dj:dj + w]
                    # DVE-pair taps share tags pa/pb, Pool-pair taps pc/pd
                    # (deeper bufs: Pool drains slowly), the last tap pe
                    ptag = {0: "pa", 1: "pb", 2: "pc", 3: "pd", 4: "pa",
                            5: "pb", 6: "pc", 7: "pd", 8: "pe"}[t]
                    pbufs = 3 if ptag in ("pc", "pd") else 2
                    pt = wkp.tile([2 * DIM, NT], BF16, tag=ptag, bufs=pbufs,
                                  name=f"pt{i}_{t}")
                    if t in ACT_TAPS:
                        qb = wkp.tile([2 * DIM, NT], BF16,
                                      tag={0: "qa", 4: "qa", 8: "qa", 1: "qd", 2: "qc", 6: "qc"}[t],
                                      bufs=2, name=f"qb{i}_{t}")
                        nc.scalar.activation(qb[:], qps[:], AF.Identity,
                                             bias=BT2[:, t:t + 1])
                        if t == 2:
                            # tap 2 feeds Pool's add chain: multiply there too
                            nc.gpsimd.tensor_mul(r3(pt[:]), r3(qb[:]), vop)
                        else:
                            nc.vector.tensor_mul(r3(pt[:]), r3(qb[:]), vop)
                    else:
                        nc.vector.scalar_tensor_tensor(
                            r3(pt[:]), r3(qps[:]),
                            BT2[:, t:t + 1], vop, op0=OP.add, op1=OP.mult)
                    pts.append(pt)
                # pairwise add tree: two pair-sums on Pool, the rest on DVE
                # two independent same-engine chains (DVE and Pool), one
                # cross-engine join at the end -- avoids per-level ping-pong
                d1 = wkp.tile([2 * DIM, NT], BF16, tag="s1", bufs=2,
                              name=f"d1_{i}")
                nc.vector.tensor_add(d1[:], pts[0][:], pts[1][:])
                c1 = wkp.tile([2 * DIM, NT], BF16, tag="s2", bufs=2,
                              name=f"c1_{i}")
                nc.gpsimd.tensor_add(c1[:], pts[2][:], pts[3][:])
                d2 = wkp.tile([2 * DIM, NT], BF16, tag="s3", bufs=2,
                              name=f"d2_{i}")
                nc.vector.tensor_add(d2[:], d1[:], pts[4][:])
                c2 = wkp.tile([2 * DIM, NT], BF16, tag="s4", bufs=2,
                              name=f"c2_{i}")
                nc.gpsimd.tensor_add(c2[:], c1[:], pts[6][:])
                d3 = wkp.tile([2 * DIM, NT], BF16, tag="u1", bufs=2,
                              name=f"d3_{i}")
                nc.vector.tensor_add(d3[:], d2[:], pts[5][:])
                c3 = wkp.tile([2 * DIM, NT], BF16, tag="u2", bufs=2,
                              name=f"c3_{i}")
                nc.gpsimd.tensor_add(c3[:], c2[:], pts[7][:])
                d4 = wkp.tile([2 * DIM, NT], BF16, tag="u3", bufs=2,
                              name=f"d4_{i}")
                nc.vector.tensor_add(d4[:], d3[:], pts[8][:])
                acc = wkp.tile([2 * DIM, NT], BF16, tag="acc", bufs=2,
                               name=f"acc{i}")
                nc.vector.tensor_add(acc[:], d4[:], c3[:])
                # lag the silu by one tile: it waits on this tile's whole DVE
                # chain, and Act must not block tile i+1's q staging on it
                if prev_acc is not None:
                    pacc, pi = prev_acc
                    nc.scalar.activation(VAGG[:, pi * NT:(pi + 1) * NT],
                                         pacc[:], AF.Silu, bias=wt["bnbd"][:],
                                         accum_out=vsums[:, pi:pi + 1])
                prev_acc = (acc, i)
            pacc, pi = prev_acc
            nc.scalar.activation(VAGG[:, pi * NT:(pi + 1) * NT], pacc[:],
                                 AF.Silu, bias=wt["bnbd"][:],
                                 accum_out=vsums[:, pi:pi + 1])

            # ---------------- SE gating ----------------
            ks = wp.tile([2 * DIM, 1], F32)
            nc.vector.reduce_sum(ks[:], ksums[:], axis=mybir.AxisListType.X)
            vs = wp.tile([2 * DIM, 1], F32)
            nc.vector.reduce_sum(vs[:], vsums[:], axis=mybir.AxisListType.X)
            g0 = wp.tile([2 * DIM, 1], F32)
            nc.vector.tensor_add(g0[:], ks[:], vs[:])
            gp0 = ps2.tile([DIM, 1], F32, tag="sm")
            nc.tensor.matmul(gp0[:], wt["fold2"][:], g0[:], start=True, stop=True)
            gap = wp.tile([DIM, 1], F32)
            nc.scalar.activation(gap[:], gp0[:], AF.Copy, scale=1.0 / HW)

            sps1 = ps2.tile([ATTN, 1], F32, tag="sm")
            nc.tensor.matmul(sps1[:], wt["sew1"][:], gap[:], start=True, stop=True)
            a1se = wp.tile([ATTN, 1], F32)
            nc.scalar.activation(a1se[:], sps1[:], AF.Relu, bias=wt["seb1"][:])
            sps2 = ps2.tile([RADIX * DIM, 1], F32, tag="sm")
            nc.tensor.matmul(sps2[:], wt["sew2"][:], a1se[:], start=True, stop=True)
            av = wp.tile([RADIX * DIM, 1], F32)
            nc.scalar.activation(av[:], sps2[:], AF.Identity, bias=wt["seb2"][:])
            sps3 = ps2.tile([DIM, 1], F32, tag="sm")
            nc.tensor.matmul(sps3[:], wt["dif"][:], av[:], start=True, stop=True)
            a0 = wp.tile([DIM, 1], F32)
            nc.scalar.activation(a0[:], sps3[:], AF.Sigmoid, bias=zbias[0:DIM, :])
            a1c = wp.tile([DIM, 1], F32)
            nc.vector.tensor_scalar(a1c[:], a0[:], -1.0, 1.0,
                                    op0=OP.mult, op1=OP.add)

            a0row = wp.tile([1, 2 * DIM], F32)
            nc.sync.dma_start(out=a0row[0:1, 0:DIM], in_=a0[:])
            nc.sync.dma_start(out=a0row[0:1, DIM:2 * DIM], in_=a0[:])
            a1row = wp.tile([1, 2 * DIM], F32)
            nc.sync.dma_start(out=a1row[0:1, 0:DIM], in_=a1c[:])
            nc.sync.dma_start(out=a1row[0:1, DIM:2 * DIM], in_=a1c[:])
            rp0 = ps2.tile([2 * DIM, 2 * DIM], F32, tag="sm")
            nc.tensor.matmul(rp0[:], wt["ones1"][:], a0row[:], start=True, stop=True)
            nc.vector.tensor_mul(diag0[:], wt["i128"][:], rp0[:])
            rp1 = ps2.tile([2 * DIM, 2 * DIM], F32, tag="sm")
            nc.tensor.matmul(rp1[:], wt["ones1"][:], a1row[:], start=True, stop=True)
            nc.vector.tensor_mul(diag1[:], wt["i128"][:], rp1[:])

            # ---------------- phase 3: blend + store ----------------
            BT = 2
            for bi in range(ntiles // BT):
                ops_ = ps2.tile([2 * DIM, BT * NT], F32, tag="bps",
                                bufs=2, name=f"ob{bi}")
                for j in range(BT):
                    i = bi * BT + j
                    sl = slice(i * NT, (i + 1) * NT)
                    psl = slice(j * NT, (j + 1) * NT)
                    nc.tensor.matmul(ops_[:, psl], diag0[:], VAGG[:, sl],
                                     start=True, stop=False)
                    nc.tensor.matmul(ops_[:, psl], diag1[:], KB[:, sl],
                                     start=False, stop=True)
                r0 = R * BT * bi
                ost = wkp.tile([2 * DIM, BT * NT], F32, tag="ost", bufs=2,
                               name=f"os{bi}")
                nc.vector.tensor_copy(ost[:], ops_[:])
                ov = ost.rearrange("p (r c) -> p r c", c=w)
                nc.sync.dma_start(out=od[:, r0:r0 + R * BT, :],
                                  in_=ov[0:DIM, :, :])
                nc.sync.dma_start(out=od[:, HH + r0:HH + r0 + R * BT, :],
                                  in_=ov[DIM:2 * DIM, :, :])

    nc.compile()
    return nc


# ---------------------------------------------------------------- entry point

_NC_CACHE = {}


def _get_nc(h, w):
    key = (h, w)
    if key not in _NC_CACHE:
        _NC_CACHE[key] = build_nc(h, w)
    return _NC_CACHE[key]


def make_in_maps(inputs, h=H, w=W):
    wts = prep_weights(inputs)
    x = np.asarray(inputs["x"], np.float32)
    maps = []
    for b in range(x.shape[0]):
        m = dict(wts)
        m["x"] = pack_x(x[b], h, w)
        maps.append(m)
    return maps


def kernel(**inputs):
    from concourse.bass_utils import run_bass_kernel_spmd
    x = np.asarray(inputs["x"], np.float32)
    b, c, h, w = x.shape
    assert b == N_CORES and c == DIM
    nc = _get_nc(h, w)
    in_maps = make_in_maps(inputs, h, w)
    res = run_bass_kernel_spmd(nc, in_maps, list(range(N_CORES)))
    out = np.stack([res.results[i]["out"] for i in range(N_CORES)], axis=0)
    return out.astype(np.float32)

